# revision 1
# baseline (speedup 1.0000x reference)
"""Trainium2 Bass kernel for nn_DenoisingLocal_Global_ConvNN_2D.

Network (per sample):
  conv3x3(3->16, pad 1) + ReLU
  -> pixel_unshuffle(2): m2 (64, 1024)  [tokens = 32x32 grid]
  -> kNN layer A: all-pairs dist on m2, top-9 (self always rank 0),
     y2 = W2_0 @ m2 + sum_{k=1..8} W2_k @ m2[:, idx_k] + b2, ReLU -> m3 (128, 1024)
  -> kNN layer B on m3: y3 = W3_0 @ m3 + sum_k W3_k @ m3[:, idx_k] + b3 (12, 1024)
  -> pixel_shuffle(2) -> (3, 64, 64)
(pixel_shuffle∘pixel_unshuffle between the two kNN layers is the identity, so
 both kNN layers run in the same 1024-token space.)

Distance trick: ranking of -dist within a row equals ranking of
A[n,j] = 2*m^T m - nsq[j]; computed as one matmul with an extra contraction row
(lhsT row = 1, rhs row = -nsq).  Self is suppressed by adding -3e38 to the
diagonal; then hardware max8/max_index give ranks 1..8 directly.

Neighbor gathers use dma_gather (wrapped-16 int16 index tables); the tables are
built with two small DRAM shuffle DMAs per layer.

Sharding: pure data parallelism, 8 samples per NeuronCore x 8 cores.
"""
import sys

for _p in ('/opt/trn_rl_repo',):
    if _p not in sys.path:
        sys.path.insert(0, _p)

import numpy as np
from contextlib import ExitStack

import concourse.bass as bass
import concourse.tile as tile
from concourse import bacc, mybir
from concourse import bass_utils

F32 = mybir.dt.float32
U16 = mybir.dt.uint16
I16 = mybir.dt.int16
AF = mybir.ActivationFunctionType

N_CORES = 8
SAMPLES = 8          # samples per core
NEG_BIG = -3.0e38

# matmul dtype knob (float32 = exact 4 cyc/row; float32r = fast, reduced prec)
DIST_DT = F32
CONV_DT = F32


def _bc(ap, dt):
    if ap.dtype == dt:
        return ap
    return ap.bitcast(dt)


# ----------------------------------------------------------------------------
# host-side input preparation (numpy)
# ----------------------------------------------------------------------------

def build_consts(w1, b1, w2, b2, w3, b3):
    w1 = np.asarray(w1, np.float32).reshape(16, 3, 3, 3)
    b1 = np.asarray(b1, np.float32)
    w2 = np.asarray(w2, np.float32).reshape(128, 64, 9)
    b2 = np.asarray(b2, np.float32)
    w3 = np.asarray(w3, np.float32).reshape(12, 128, 9)
    b3 = np.asarray(b3, np.float32)

    # conv1 lhsT: 4 phases, K=28 (27 taps + bias row), M=64 (16 ch x 4 phases)
    c1 = np.zeros((4, 28, 64), np.float32)
    for q in range(4):
        for dy in range(3):
            for dx in range(3):
                c1[q, np.arange(3)[:, None] * 9 + dy * 3 + dx,
                   np.arange(16)[None, :] * 4 + q] = w1[:, :, dy, dx].T
        c1[q, 27, np.arange(16) * 4 + q] = b1
    c1 = np.ascontiguousarray(c1.transpose(1, 0, 2).reshape(28, 256))

    # W2 neighbor pair K-tiles: (128, 4*128); pair P cols [128P:128P+128]
    # K-tile rows [64h + c] hold tap (2P + h + 1)
    w2pairs = np.zeros((128, 512), np.float32)
    for P in range(4):
        for h in range(2):
            w2pairs[64 * h:64 * h + 64, 128 * P:128 * P + 128] = \
                w2[:, :, 2 * P + h + 1].T
    w2self = np.zeros((65, 128), np.float32)
    w2self[:64] = w2[:, :, 0].T
    w2self[64] = b2

    # output-channel permutation: co = ch*4+q -> co' = q*3+ch so each
    # pixel_shuffle phase q reads contiguous partitions [3q:3q+3]
    perm = np.zeros(12, np.int64)
    for ch in range(3):
        for q in range(4):
            perm[q * 3 + ch] = ch * 4 + q
    w3 = w3[perm]
    b3 = b3[perm]

    # W3 zcat lhsT (128, 96): col 12k+co'
    w3zcat = np.zeros((128, 96), np.float32)
    for k in range(8):
        w3zcat[:, 12 * k:12 * k + 12] = w3[:, :, k + 1].T
    w3self = np.ascontiguousarray(w3[:, :, 0].T)          # (128, 12)
    b3col = np.ascontiguousarray(b3[:, None])             # (12, 1)

    ident = np.eye(128, dtype=np.float32)
    diagneg = np.zeros((128, 128), np.float32)
    np.fill_diagonal(diagneg, NEG_BIG)

    return dict(c1=c1, w2pairs=w2pairs, w2self=w2self,
                w3zcat=w3zcat, w3self=w3self, b3col=b3col, ident=ident,
                diagneg=diagneg)


def build_p27(x_shard):
    """Per-phase im2col for conv1: (S, 4, 28, 1024).
    p27[s, q=(sy,sx), 9ci+3dy+dx, 32y+x] = xpad[s, ci, 2y+sy+dy, 2x+sx+dx];
    row 27 = 1.0 (bias)."""
    S = x_shard.shape[0]
    xp = np.zeros((S, 3, 66, 66), np.float32)
    xp[:, :, 1:65, 1:65] = x_shard
    p27 = np.ones((S, 4, 28, 1024), np.float32)
    for q in range(4):
        sy, sx = q >> 1, q & 1
        for ci in range(3):
            for dy in range(3):
                for dx in range(3):
                    v = xp[:, ci, sy + dy:sy + dy + 64:2, sx + dx:sx + dx + 64:2]
                    p27[:, q, ci * 9 + dy * 3 + dx, :] = v.reshape(S, 1024)
    return p27


# ----------------------------------------------------------------------------
# device program
# ----------------------------------------------------------------------------

def _ap(base_ap, offset, dims):
    return bass.AP(base_ap.tensor, offset, [list(d) for d in dims])


def _emit_wrapped_idx(nc, dp, sp, idxv, tag):
    """idxv: SBUF (128, 64) u16 AP, value for slot s = blk*128 + p at [p, blk].
    Produces the wrapped dma_gather table (128, 512) u16:
      W[16r + b, 8*blk + a] = idxv[16a + b, blk]   (replicated over r)
    via two DRAM hops (all DMA APs <= 3 dims)."""
    da = dp.tile([8192], U16, tag=tag + "a")
    # plain dump: da flat = p*64 + blk = a*1024 + b*64 + blk (p = 16a + b)
    nc.sync.dma_start(da[:], idxv)
    db = dp.tile([8192], U16, tag=tag + "b")
    # shuffle: dst flat = b*512 + blk*8 + a <- src a*1024 + b*64 + blk
    nc.sync.dma_start(
        _ap(db[:], 0, [[512, 16], [8, 64], [1, 8]]),
        _ap(da[:], 0, [[64, 16], [1, 64], [1024, 8]]))
    w = sp.tile([128, 512], U16, tag=tag + "w")
    # replicate into (16r + b, c) <- src b*512 + c
    nc.sync.dma_start(
        w[:], _ap(db[:], 0, [[0, 8], [512, 16], [1, 512]]))
    return w


def build_program(nc, samples=SAMPLES, tap=None, stage=99, repeat=1):
    p27_d = nc.dram_tensor("p27", (samples, 4, 28, 1024), F32, kind="ExternalInput").ap()
    c1_d = nc.dram_tensor("c1", (28, 256), F32, kind="ExternalInput").ap()
    w2p_d = nc.dram_tensor("w2pairs", (128, 512), F32, kind="ExternalInput").ap()
    w2s_d = nc.dram_tensor("w2self", (65, 128), F32, kind="ExternalInput").ap()
    w3z_d = nc.dram_tensor("w3zcat", (128, 96), F32, kind="ExternalInput").ap()
    w3s_d = nc.dram_tensor("w3self", (128, 12), F32, kind="ExternalInput").ap()
    b3_d = nc.dram_tensor("b3col", (12, 1), F32, kind="ExternalInput").ap()
    id_d = nc.dram_tensor("ident", (128, 128), F32, kind="ExternalInput").ap()
    dg_d = nc.dram_tensor("diagneg", (128, 128), F32, kind="ExternalInput").ap()
    out_d = nc.dram_tensor("out", (samples, 12, 1024), F32, kind="ExternalOutput").ap()

    if tap is None:
        def tap(name, t):
            pass

    with tile.TileContext(nc) as tc, ExitStack() as ctx:
        cp = ctx.enter_context(tc.tile_pool(name="consts", bufs=1))
        sp = ctx.enter_context(tc.tile_pool(name="sb", bufs=2))
        bp = ctx.enter_context(tc.tile_pool(name="big", bufs=1))
        gp = ctx.enter_context(tc.tile_pool(name="gm", bufs=5))
        pp = ctx.enter_context(tc.tile_pool(name="ps", bufs=4, space="PSUM"))
        dp = ctx.enter_context(tc.tile_pool(name="dram", bufs=2, space="DRAM"))

        c1 = cp.tile([28, 256], F32); nc.sync.dma_start(c1[:], c1_d)
        w2p = cp.tile([128, 512], F32); nc.sync.dma_start(w2p[:], w2p_d)
        w2s = cp.tile([65, 128], F32); nc.sync.dma_start(w2s[:], w2s_d)
        w3z = cp.tile([128, 96], F32); nc.sync.dma_start(w3z[:], w3z_d)
        w3s = cp.tile([128, 12], F32); nc.sync.dma_start(w3s[:], w3s_d)
        b3c = cp.tile([12, 1], F32); nc.sync.dma_start(b3c[:], b3_d)
        ident = cp.tile([128, 128], F32); nc.sync.dma_start(ident[:], id_d)
        diag = cp.tile([128, 128], F32); nc.sync.dma_start(diag[:], dg_d)
        ones64 = cp.tile([64, 1], F32); nc.gpsimd.memset(ones64[:], 1.0)
        ones128 = cp.tile([128, 1], F32); nc.gpsimd.memset(ones128[:], 1.0)
        onesr = cp.tile([1, 128], F32); nc.gpsimd.memset(onesr[:], 1.0)

        for s in [ss for _ in range(repeat) for ss in range(samples)]:
            # ================= conv1 =================
            p27 = bp.tile([28, 4096], F32, tag="p27")
            nc.sync.dma_start(p27[:].rearrange("p (q n) -> p q n", q=4),
                              p27_d[s].rearrange("q p n -> p q n"))
            H = pp.tile([128, 1024], F32, tag="ps")
            for cch in range(2):
                for q in range(4):
                    rhs = p27[:, 1024 * q + 512 * cch:1024 * q + 512 * cch + 512]
                    nc.tensor.matmul(H[:64, 512 * cch:512 * cch + 512],
                                     _bc(c1[:, 64 * q:64 * q + 64], CONV_DT),
                                     _bc(rhs, CONV_DT), start=(q == 0), stop=(q == 3))
            m2l = sp.tile([65, 1024], F32, tag="m2l")
            nc.scalar.activation(m2l[:64, :], H[:64, :], AF.Relu)
            nc.gpsimd.memset(m2l[64:65, :], 1.0)
            m2r = sp.tile([65, 1024], F32, tag="m2r")
            nc.scalar.activation(m2r[:64, :], H[:64, :], AF.Relu, scale=2.0)
            msq = sp.tile([64, 1024], F32, tag="msq")
            nc.scalar.activation(msq[:], m2l[:64, :], AF.Square)
            nsqp = pp.tile([128, 1024], F32, tag="ps")
            for cch in range(2):
                nc.tensor.matmul(nsqp[:1, 512 * cch:512 * cch + 512], ones64[:],
                                 msq[:, 512 * cch:512 * cch + 512],
                                 start=True, stop=True)
            nc.scalar.activation(m2r[64:65, :], nsqp[:1, :], AF.Copy, scale=-1.0)
            tap("m2l_%d" % s, m2l[:])
            tap("m2r_%d" % s, m2r[:])

            # mT2 to DRAM for the neighbor gather
            ttp = pp.tile([128, 1024], F32, tag="ps")
            for t in range(8):
                nc.tensor.matmul(ttp[:, 64 * t:64 * t + 64],
                                 m2l[:64, 128 * t:128 * t + 128],
                                 ident[:64, :64], is_transpose=True,
                                 start=True, stop=True)
            mt2sb = sp.tile([128, 512], F32, tag="mt2")
            nc.scalar.activation(mt2sb[:], ttp[:, :512], AF.Copy)
            mt2_dram = dp.tile([1024, 64], F32, tag="mt2d")
            nc.sync.dma_start(
                mt2_dram[:].rearrange("(t p) c -> p t c", p=128),
                mt2sb[:].rearrange("p (t c) -> p t c", t=8))
            tap("mt2_%d" % s, mt2_dram[:])
            if stage < 2:
                continue

            # ================= layer A kNN =================
            idxn = sp.tile([128, 8, 8], U16, tag="idxn")   # [p][t][k]
            vals2 = sp.tile([128, 8, 8], F32, tag="vals2")
            for t in range(8):
                A = pp.tile([128, 1024], F32, tag="ps")
                for cch in range(2):
                    nc.tensor.matmul(A[:, 512 * cch:512 * cch + 512],
                                     _bc(m2l[:, 128 * t:128 * t + 128], DIST_DT),
                                     _bc(m2r[:, 512 * cch:512 * cch + 512], DIST_DT),
                                     start=True, stop=True)
                nc.vector.tensor_add(A[:, 128 * t:128 * t + 128],
                                     A[:, 128 * t:128 * t + 128], diag[:])
                nc.vector.max(vals2[:, t, :], A[:])
                nc.vector.max_index(idxn[:, t, :], vals2[:, t, :], A[:])
            tap("idxn_%d" % s, idxn[:])
            if stage < 3:
                continue

            # permute [p][t][k] -> [p][blk] with blk = 16P + 2t + h, k = 2P+h
            idxv2 = sp.tile([128, 64], U16, tag="idxv2")
            nc.vector.tensor_copy(
                idxv2[:].rearrange("p (P t h) -> p P t h", P=4, t=8),
                idxn[:].rearrange("p t (P h) -> p P t h", P=4))
            w2idx = _emit_wrapped_idx(nc, dp, sp, idxv2[:], "i2")
            g2 = bp.tile([128, 4, 8, 2, 64], F32, tag="g2")  # [p][P][t][h][c]
            nc.gpsimd.dma_gather(
                g2[:].rearrange("p P t h c -> p (P t h) c"),
                mt2_dram[:], _bc(w2idx[:], I16), 8192, 8192, 64,
                single_packet=False)
            tap("g2_%d" % s, g2[:].rearrange("p P t h c -> p (P t h) c"))

            # transpose pairs -> feature-major K-tiles, then conv
            y2 = pp.tile([128, 1024], F32, tag="ps")
            gmats = []
            for P in range(4):
                gtp = pp.tile([128, 1024], F32, tag="ps")
                for t in range(8):
                    nc.tensor.matmul(
                        gtp[:, 128 * t:128 * t + 128],
                        g2[:, P, t, :, :].rearrange("p h c -> p (h c)"),
                        ident[:], is_transpose=True, start=True, stop=True)
                gm = gp.tile([128, 1024], F32, tag="gmat")
                nc.scalar.activation(gm[:], gtp[:], AF.Copy)
                gmats.append(gm)
            for cch in range(2):
                sl = slice(512 * cch, 512 * cch + 512)
                for P in range(4):
                    nc.tensor.matmul(y2[:, sl],
                                     _bc(w2p[:, 128 * P:128 * P + 128], CONV_DT),
                                     _bc(gmats[P][:, sl], CONV_DT),
                                     start=(P == 0), stop=False)
                nc.tensor.matmul(y2[:, sl], _bc(w2s[:], CONV_DT),
                                 _bc(m2l[:, sl], CONV_DT), start=False, stop=True)

            if stage < 4:
                continue
            # ================= layer B prep =================
            m3 = sp.tile([128, 1024], F32, tag="m3")
            nc.scalar.activation(m3[:], y2[:], AF.Relu)
            tap("m3_%d" % s, m3[:])
            m3r = sp.tile([128, 1024], F32, tag="m3r")
            nc.scalar.activation(m3r[:], y2[:], AF.Relu, scale=2.0)
            msq3 = sp.tile([128, 1024], F32, tag="msq3")
            nc.scalar.activation(msq3[:], m3[:], AF.Square)
            nsq3p = pp.tile([128, 1024], F32, tag="ps")
            for cch in range(2):
                nc.tensor.matmul(nsq3p[:1, 512 * cch:512 * cch + 512], ones128[:],
                                 msq3[:, 512 * cch:512 * cch + 512],
                                 start=True, stop=True)
            nsq3n = sp.tile([1, 1024], F32, tag="nsq3")
            nc.scalar.activation(nsq3n[:], nsq3p[:1, :], AF.Copy, scale=-1.0)

            # Zcat (96, 1024) = taps 1..8 of W3 @ m3; ZT (1024, 128-pad) to DRAM
            zcp = pp.tile([128, 1024], F32, tag="ps")
            for cch in range(2):
                nc.tensor.matmul(zcp[:96, 512 * cch:512 * cch + 512],
                                 _bc(w3z[:], CONV_DT),
                                 _bc(m3[:, 512 * cch:512 * cch + 512], CONV_DT),
                                 start=True, stop=True)
            zcs = sp.tile([96, 1024], F32, tag="zcs")
            nc.scalar.activation(zcs[:], zcp[:96, :], AF.Copy)
            # rows padded to 128 f32 (512B) for dma_gather alignment
            ztsb = sp.tile([128, 8, 128], F32, tag="ztsb")
            nc.gpsimd.memset(ztsb[:, :, 96:], 0.0)
            for g in range(2):
                ztp = pp.tile([128, 1024], F32, tag="ps")
                for tt in range(4):
                    t = 4 * g + tt
                    nc.tensor.matmul(ztp[:, 96 * tt:96 * tt + 96],
                                     zcs[:, 128 * t:128 * t + 128],
                                     ident[:96, :96], is_transpose=True,
                                     start=True, stop=True)
                nc.scalar.activation(
                    _ap(ztsb[:], 4 * g * 128, [[1024, 128], [128, 4], [1, 96]]),
                    ztp[:, :384], AF.Copy)
            zt3_dram = dp.tile([1024, 128], F32, tag="zt3d")
            nc.sync.dma_start(
                zt3_dram[:].rearrange("(t p) c -> p t c", p=128),
                ztsb[:])
            tap("zt3_%d" % s, zt3_dram[:])

            if stage < 5:
                continue
            # ================= layer B kNN =================
            idx3 = sp.tile([128, 8, 8], U16, tag="idx3")   # [p][t][k]; blk = 8t+k
            vals3 = sp.tile([128, 8, 8], F32, tag="vals3")
            for t in range(8):
                A = pp.tile([128, 1024], F32, tag="ps")
                for cch in range(2):
                    sl = slice(512 * cch, 512 * cch + 512)
                    nc.tensor.matmul(A[:, sl],
                                     _bc(m3[:, 128 * t:128 * t + 128], DIST_DT),
                                     _bc(m3r[:, sl], DIST_DT), start=True, stop=False)
                    nc.tensor.matmul(A[:, sl], _bc(onesr[:], DIST_DT),
                                     _bc(nsq3n[:, sl], DIST_DT),
                                     start=False, stop=True)
                nc.vector.tensor_add(A[:, 128 * t:128 * t + 128],
                                     A[:, 128 * t:128 * t + 128], diag[:])
                nc.vector.max(vals3[:, t, :], A[:])
                nc.vector.max_index(idx3[:, t, :], vals3[:, t, :], A[:])
            tap("idx3_%d" % s, idx3[:])

            if stage < 6:
                continue
            w3idx = _emit_wrapped_idx(nc, dp, sp,
                                      idx3[:].rearrange("p t k -> p (t k)"), "i3")
            g3 = bp.tile([128, 64, 128], F32, tag="g3")    # [p][blk=8t+k][128]
            nc.gpsimd.dma_gather(
                g3[:], zt3_dram[:], _bc(w3idx[:], I16), 8192, 8192, 128,
                single_packet=False)
            tap("g3_%d" % s, g3[:])

            if stage < 7:
                continue
            # r3[p, t, c] = sum_k g3[p, 8t+k, 12k + c]
            r3 = sp.tile([128, 8, 12], F32, tag="r3")
            red_in = _ap(g3[:], 0, [[8192, 128], [1024, 8], [1, 12], [140, 8]])
            nc.vector.tensor_reduce(r3[:], red_in, axis=mybir.AxisListType.X,
                                    op=mybir.AluOpType.add)
            tap("r3_%d" % s, r3[:])

            # y3 = W3_0 @ m3 + bias + neighbor sums (transposed back)
            y3p = pp.tile([128, 1024], F32, tag="ps")
            for cch in range(2):
                nc.tensor.matmul(y3p[:12, 512 * cch:512 * cch + 512],
                                 _bc(w3s[:], CONV_DT),
                                 _bc(m3[:, 512 * cch:512 * cch + 512], CONV_DT),
                                 start=True, stop=False)
            for t in range(8):
                nc.tensor.matmul(y3p[:12, 128 * t:128 * t + 128],
                                 r3[:, t, :], ident[:], is_transpose=True,
                                 start=False, stop=(t % 4 == 3))
            y3sb = sp.tile([12, 1024], F32, tag="y3")
            nc.scalar.activation(y3sb[:], y3p[:12, :], AF.Identity, bias=b3c[:])
            tap("y3_%d" % s, y3sb[:])

            # contiguous store; pixel_shuffle happens on the host
            nc.sync.dma_start(out_d[s], y3sb[:])

    return nc


_CACHE = {}


def _get_compiled():
    if 'nc' not in _CACHE:
        nc = bacc.Bacc("TRN2", target_bir_lowering=False, debug=False,
                       num_devices=N_CORES)
        build_program(nc, SAMPLES)
        nc.compile()
        _CACHE['nc'] = nc
    return _CACHE['nc']


def make_in_maps(x, consts):
    in_maps = []
    for c in range(N_CORES):
        shard = np.ascontiguousarray(x[c * SAMPLES:(c + 1) * SAMPLES],
                                     dtype=np.float32)
        m = dict(consts)
        m['p27'] = build_p27(shard).astype(np.float32)
        in_maps.append(m)
    return in_maps


def kernel(x, conv1_w, conv1_b, conv2_w, conv2_b, conv3_w, conv3_b, **_ignored):
    x = np.asarray(x, np.float32)
    consts = build_consts(conv1_w, conv1_b, conv2_w, conv2_b, conv3_w, conv3_b)
    nc = _get_compiled()
    in_maps = make_in_maps(x, consts)
    res = bass_utils.run_bass_kernel_spmd(nc, in_maps, core_ids=list(range(N_CORES)))
    y3 = np.concatenate([res.results[c]['out'] for c in range(N_CORES)], axis=0)
    return shuffle_out(y3)


def shuffle_out(y3):
    """y3 (B, 12, 1024) with channel rows co' = q*3+ch -> (B, 3, 64, 64)."""
    B = y3.shape[0]
    y = y3.reshape(B, 4, 3, 32, 32)                # [b][q=(sy,sx)][ch][h][w]
    out = np.zeros((B, 3, 64, 64), np.float32)
    for q in range(4):
        sy, sx = q >> 1, q & 1
        out[:, :, sy::2, sx::2] = y[:, q]
    return out.astype(np.float32)


if __name__ == '__main__':
    nc = _get_compiled()
    print("compiled ok")



# revision 12
# speedup vs baseline: 1.7095x; 1.7095x over previous
"""Trainium2 Bass kernel for nn_DenoisingLocal_Global_ConvNN_2D (v2).

Network (per sample):
  conv3x3(3->16, pad 1) + ReLU
  -> pixel_unshuffle(2): m2 (64, 1024)  [tokens = 32x32 grid]
  -> kNN layer A: all-pairs dist on m2, top-9 (self rank 0),
     y2 = W2_0 @ m2 + sum_{k=1..8} W2_k @ m2[:, idx_k] + b2, ReLU -> m3 (128,1024)
  -> kNN layer B on m3: y3 = W3_0 @ m3 + sum_k W3_k @ m3[:, idx_k] + b3 (12,1024)
  -> pixel_shuffle(2) -> (3, 64, 64)

v2 design vs v1:
  * SBUF-source transposed dma_gather (tokens_per_rank=128): source layout =
    PE-transpose of the feature matrix (token c at partition c%128, rank
    c//128), gathered output is directly feature-major [ch, slot] bf16 --
    no DRAM round trip, no per-pair PE un-transposes.
  * Layer A gathers hi/lo bf16 split of m2 (exact to ~2^-18); layer B gathers
    plain bf16 m3.
  * Wrapped idx table build keeps the 3-DMA chain but with a slot order
    (slot = 1024a + 128k + 16t + b for token n=128t+16a+b) that makes every
    DMA hop move 128-byte-contiguous runs (v1's middle hop was 2-byte runs).
    Side effect: y2/m3 columns are in pi(n)=128a+16t+b order (an involution);
    layer B runs entirely in pi-space and y3 comes out in ORIGINAL order.
  * All matmuls fp32r (1 cyc/col at N=512) or bf16 instead of fp32 (4 cyc).

Sharding: pure data parallelism, 8 samples per NeuronCore x 8 cores.
"""
import sys

for _p in ('/opt/trn_rl_repo',):
    if _p not in sys.path:
        sys.path.insert(0, _p)

import numpy as np
import ml_dtypes
from contextlib import ExitStack

import concourse.bass as bass
import concourse.tile as tile
from concourse import bacc, mybir
from concourse import bass_utils

F32 = mybir.dt.float32
F32R = mybir.dt.float32r
BF16 = mybir.dt.bfloat16
U16 = mybir.dt.uint16
I16 = mybir.dt.int16
AF = mybir.ActivationFunctionType
NPBF16 = ml_dtypes.bfloat16

N_CORES = 8
SAMPLES = 8          # samples per core
NEG_BIG = -3.0e38

DIST_DT = F32        # distance matmuls (ranking needs full fp32)
CONV_DT = F32        # fp32-data conv matmuls (feed the ranking path)


def _bc(ap, dt):
    if ap.dtype == dt:
        return ap
    return ap.bitcast(dt)


# ----------------------------------------------------------------------------
# host-side input preparation (numpy)
# ----------------------------------------------------------------------------

def build_consts(w1, b1, w2, b2, w3, b3):
    w1 = np.asarray(w1, np.float32).reshape(16, 3, 3, 3)
    b1 = np.asarray(b1, np.float32)
    w2 = np.asarray(w2, np.float32).reshape(128, 64, 9)
    b2 = np.asarray(b2, np.float32)
    w3 = np.asarray(w3, np.float32).reshape(12, 128, 9)
    b3 = np.asarray(b3, np.float32)

    # conv1 lhsT: 4 phases, K=28 (27 taps + bias row), M=64 (16 ch x 4 phases)
    c1 = np.zeros((4, 28, 64), np.float32)
    for q in range(4):
        for dy in range(3):
            for dx in range(3):
                c1[q, np.arange(3)[:, None] * 9 + dy * 3 + dx,
                   np.arange(16)[None, :] * 4 + q] = w1[:, :, dy, dx].T
        c1[q, 27, np.arange(16) * 4 + q] = b1
    c1 = np.ascontiguousarray(c1.transpose(1, 0, 2).reshape(28, 256))

    # W2 neighbor taps, hi/lo split for full fp32 weight precision.
    # wh/wl bf16 [128, 8*128]: tap k block k-1; rows 0..63 == rows 64..127
    # (gathered elements are [hi|lo] splits, both halves see the same W part)
    w2f = np.stack([w2[:, :, k].T for k in range(1, 9)], 0)   # (8, 64, 128)
    w2hi = w2f.astype(NPBF16).astype(np.float32)
    w2lo = (w2f - w2hi).astype(NPBF16)
    w2hi = w2hi.astype(NPBF16)
    w2tapsA = np.zeros((128, 8, 128), NPBF16)
    w2tapsB = np.zeros((128, 8, 128), NPBF16)
    for k in range(8):
        w2tapsA[:64, k] = w2hi[k]
        w2tapsA[64:, k] = w2hi[k]
        w2tapsB[:64, k] = w2lo[k]
        w2tapsB[64:, k] = w2lo[k]
    w2tapsA = w2tapsA.reshape(128, 1024)
    w2tapsB = w2tapsB.reshape(128, 1024)
    w2self = np.zeros((65, 128), np.float32)
    w2self[:64] = w2[:, :, 0].T
    w2self[64] = b2

    # output-channel permutation: co = ch*4+q -> co' = q*3+ch so each
    # pixel_shuffle phase q reads contiguous partitions [3q:3q+3]
    perm = np.zeros(12, np.int64)
    for ch in range(3):
        for q in range(4):
            perm[q * 3 + ch] = ch * 4 + q
    w3 = w3[perm]
    b3 = b3[perm]

    # W3 neighbor taps, bf16 [128, 8*12]
    w3taps = np.zeros((128, 8, 12), np.float32)
    for k in range(1, 9):
        w3taps[:, k - 1] = w3[:, :, k].T
    w3taps = w3taps.reshape(128, 96).astype(NPBF16)
    w3self = np.ascontiguousarray(w3[:, :, 0].T)          # (128, 12)
    b3col = np.ascontiguousarray(b3[:, None])             # (12, 1)

    ident = np.eye(128, dtype=np.float32)
    diagneg = np.zeros((128, 128), np.float32)
    np.fill_diagonal(diagneg, NEG_BIG)

    return dict(c1=c1, w2tapsA=w2tapsA, w2tapsB=w2tapsB, w2self=w2self,
                w3taps=w3taps, w3self=w3self, b3col=b3col, ident=ident,
                diagneg=diagneg)


def build_p27(x_shard):
    """Per-phase im2col for conv1: (S, 4, 28, 1024).
    p27[s, q=(sy,sx), 9ci+3dy+dx, 32y+x] = xpad[s, ci, 2y+sy+dy, 2x+sx+dx];
    row 27 = 1.0 (bias)."""
    S = x_shard.shape[0]
    xp = np.zeros((S, 3, 66, 66), np.float32)
    xp[:, :, 1:65, 1:65] = x_shard
    p27 = np.ones((S, 4, 28, 1024), np.float32)
    for q in range(4):
        sy, sx = q >> 1, q & 1
        for ci in range(3):
            for dy in range(3):
                for dx in range(3):
                    v = xp[:, ci, sy + dy:sy + dy + 64:2, sx + dx:sx + dx + 64:2]
                    p27[:, q, ci * 9 + dy * 3 + dx, :] = v.reshape(S, 1024)
    return p27


# ----------------------------------------------------------------------------
# device program
# ----------------------------------------------------------------------------

def _ap(base_ap, offset, dims):
    return bass.AP(base_ap.tensor, offset, [list(d) for d in dims])


def _emit_idx_table(nc, dp, sp, idxv, tag):
    """idxv: SBUF (128, 64) u16, value at [p=16a+b, m] is the index for
    slot s = 16*(64a + m) + b.  Produces the wrapped dma_gather table
    (128, 512) u16: W[16r + b, j] = value(slot 16j + b), j = 64a + m.
    All three DMA hops move >=128B-contiguous runs."""
    da = dp.tile([8192], U16, tag=tag + "a")
    # dump: da flat = p*64 + m = 1024a + 64b + m
    nc.sync.dma_start(da[:], idxv)
    db = dp.tile([8192], U16, tag=tag + "b")
    # shuffle: db[b*512 + 64a + m] <- da[1024a + 64b + m]  (64-elem runs)
    nc.sync.dma_start(
        _ap(db[:], 0, [[512, 16], [64, 8], [1, 64]]),
        _ap(da[:], 0, [[64, 16], [1024, 8], [1, 64]]))
    w = sp.tile([128, 512], U16, tag=tag + "w")
    # replicate rows b -> 16r+b
    nc.sync.dma_start(
        w[:], _ap(db[:], 0, [[0, 8], [512, 16], [1, 512]]))
    return w


def _pi_rhs(t_ap, part, ncols_off_h):
    """rhs AP streaming columns of a [part, 1024] SBUF tile in pi order
    (a, t, b) for cch half h: offset 16*(4h), dims a(4),t(8),b(16)."""
    h = ncols_off_h
    return _ap(t_ap, 16 * 4 * h,
               [[1024, part], [16, 4], [128, 8], [1, 16]])


def build_program(nc, samples=SAMPLES, tap=None, stage=99, repeat=1):
    p27_d = nc.dram_tensor("p27", (samples, 4, 28, 1024), F32, kind="ExternalInput").ap()
    c1_d = nc.dram_tensor("c1", (28, 256), F32, kind="ExternalInput").ap()
    w2tA_d = nc.dram_tensor("w2tapsA", (128, 1024), BF16, kind="ExternalInput").ap()
    w2tB_d = nc.dram_tensor("w2tapsB", (128, 1024), BF16, kind="ExternalInput").ap()
    w2s_d = nc.dram_tensor("w2self", (65, 128), F32, kind="ExternalInput").ap()
    w3t_d = nc.dram_tensor("w3taps", (128, 96), BF16, kind="ExternalInput").ap()
    w3s_d = nc.dram_tensor("w3self", (128, 12), F32, kind="ExternalInput").ap()
    b3_d = nc.dram_tensor("b3col", (12, 1), F32, kind="ExternalInput").ap()
    id_d = nc.dram_tensor("ident", (128, 128), F32, kind="ExternalInput").ap()
    dg_d = nc.dram_tensor("diagneg", (128, 128), F32, kind="ExternalInput").ap()
    out_d = nc.dram_tensor("out", (samples, 12, 1024), F32, kind="ExternalOutput").ap()

    if tap is None:
        def tap(name, t):
            pass

    with tile.TileContext(nc) as tc, ExitStack() as ctx:
        cp = ctx.enter_context(tc.tile_pool(name="consts", bufs=1))
        sp = ctx.enter_context(tc.tile_pool(name="sb", bufs=2))
        gp = ctx.enter_context(tc.tile_pool(name="gm", bufs=2))
        ppA = ctx.enter_context(tc.tile_pool(name="psA", bufs=2, space="PSUM"))
        ppa = ctx.enter_context(tc.tile_pool(name="psacc", bufs=2, space="PSUM"))
        dp = ctx.enter_context(tc.tile_pool(name="dram", bufs=2, space="DRAM"))

        c1 = cp.tile([28, 256], F32); nc.sync.dma_start(c1[:], c1_d)
        w2tA = cp.tile([128, 1024], BF16); nc.sync.dma_start(w2tA[:], w2tA_d)
        w2tB = cp.tile([128, 1024], BF16); nc.sync.dma_start(w2tB[:], w2tB_d)
        w2s = cp.tile([65, 128], F32); nc.sync.dma_start(w2s[:], w2s_d)
        w3t = cp.tile([128, 96], BF16); nc.sync.dma_start(w3t[:], w3t_d)
        w3s = cp.tile([128, 12], F32); nc.sync.dma_start(w3s[:], w3s_d)
        b3c = cp.tile([12, 1], F32); nc.sync.dma_start(b3c[:], b3_d)
        ident = cp.tile([128, 128], F32); nc.sync.dma_start(ident[:], id_d)
        diag = cp.tile([128, 128], F32); nc.sync.dma_start(diag[:], dg_d)
        ones64 = cp.tile([64, 1], F32); nc.gpsimd.memset(ones64[:], 1.0)
        ones128 = cp.tile([128, 1], F32); nc.gpsimd.memset(ones128[:], 1.0)
        onesr = cp.tile([1, 128], F32); nc.gpsimd.memset(onesr[:], 1.0)

        for s in [ss for _ in range(repeat) for ss in range(samples)]:
            # ================= conv1 =================
            p27 = sp.tile([28, 4096], F32, tag="p27")
            nc.sync.dma_start(p27[:].rearrange("p (q n) -> p q n", q=4),
                              p27_d[s].rearrange("q p n -> p q n"))
            H = ppa.tile([128, 1024], F32, tag="acc")
            for cch in range(2):
                for q in range(4):
                    rhs = p27[:, 1024 * q + 512 * cch:1024 * q + 512 * cch + 512]
                    nc.tensor.matmul(H[:64, 512 * cch:512 * cch + 512],
                                     _bc(c1[:, 64 * q:64 * q + 64], CONV_DT),
                                     _bc(rhs, CONV_DT), start=(q == 0), stop=(q == 3))
            m2l = sp.tile([65, 1024], F32, tag="m2l")
            nc.scalar.activation(m2l[:64, :], H[:64, :], AF.Relu)
            nc.gpsimd.memset(m2l[64:65, :], 1.0)
            m2r = sp.tile([65, 1024], F32, tag="m2r")
            nc.scalar.activation(m2r[:64, :], H[:64, :], AF.Relu, scale=2.0)
            msq = sp.tile([64, 1024], F32, tag="msq")
            nc.scalar.activation(msq[:], m2l[:64, :], AF.Square)
            nsqp = ppa.tile([1, 1024], F32, tag="acc")
            for cch in range(2):
                nc.tensor.matmul(nsqp[:1, 512 * cch:512 * cch + 512],
                                 _bc(ones64[:], CONV_DT),
                                 _bc(msq[:, 512 * cch:512 * cch + 512], CONV_DT),
                                 start=True, stop=True)
            nc.scalar.activation(m2r[64:65, :], nsqp[:1, :], AF.Copy, scale=-1.0)
            tap("m2l_%d" % s, m2l[:])
            tap("m2r_%d" % s, m2r[:])

            # m2T: token-major gather source [p, rank, 128] bf16 = [hi | lo]
            ttp = ppA.tile([128, 1024], F32, tag="A")
            for t in range(8):
                nc.tensor.matmul(ttp[:, 64 * t:64 * t + 64],
                                 m2l[:64, 128 * t:128 * t + 128],
                                 ident[:64, :64], is_transpose=True,
                                 start=True, stop=True)
            m2T = sp.tile([128, 8, 128], BF16, tag="m2T")
            nc.scalar.activation(
                _ap(m2T[:], 0, [[1024, 128], [128, 8], [1, 64]]),
                ttp[:, :512], AF.Copy)
            nc.vector.tensor_sub(
                _ap(m2T[:], 64, [[1024, 128], [128, 8], [1, 64]]),
                ttp[:, :512],
                _ap(m2T[:], 0, [[1024, 128], [128, 8], [1, 64]]))
            tap("m2T_%d" % s, m2T[:])
            if stage < 2:
                continue

            # ================= layer A kNN =================
            idxn = sp.tile([128, 8, 8], U16, tag="idxn")   # [p][t][k]
            vals2 = sp.tile([128, 8, 8], F32, tag="vals2")
            for t in range(8):
                A = ppA.tile([128, 1024], F32, tag="A")
                for cch in range(2):
                    nc.tensor.matmul(A[:, 512 * cch:512 * cch + 512],
                                     _bc(m2l[:, 128 * t:128 * t + 128], DIST_DT),
                                     _bc(m2r[:, 512 * cch:512 * cch + 512], DIST_DT),
                                     start=True, stop=True)
                nc.vector.tensor_add(A[:, 128 * t:128 * t + 128],
                                     A[:, 128 * t:128 * t + 128], diag[:])
                nc.vector.max(vals2[:, t, :], A[:])
                nc.vector.max_index(idxn[:, t, :], vals2[:, t, :], A[:])
            tap("idxn_%d" % s, idxn[:])
            if stage < 3:
                continue

            # idxv[p, m=8k+t] = idxn[p, t, k]
            idxv2 = sp.tile([128, 64], U16, tag="idxv2")
            nc.vector.tensor_copy(
                idxv2[:].rearrange("p (k t) -> p k t", k=8),
                idxn[:].rearrange("p t k -> p k t"))
            w2idx = _emit_idx_table(nc, dp, sp, idxv2[:], "i2")
            g2 = gp.tile([128, 8192], BF16, tag="g2")
            nc.gpsimd.dma_gather(
                g2[:].rearrange("p (o n) -> p o n", o=1),
                m2T[:].rearrange("p r c -> p (r c)"),
                _bc(w2idx[:], I16), 8192, 8192, 128,
                transpose=True, single_packet=False,
                sbuf_tokens_per_rank=128, sbuf_free_dim_per_rank=256)
            tap("g2_%d" % s, g2[:])
            if stage < 4:
                continue

            # y2 = W2self@m2l(pi-order) + sum_k W2k @ g2[tap k]
            y2 = ppa.tile([128, 1024], F32, tag="acc")
            for h in range(2):
                sl = slice(512 * h, 512 * h + 512)
                for k in range(8):
                    rhs = _ap(g2[:], 4096 * h + 128 * k,
                              [[8192, 128], [1024, 4], [1, 128]])
                    nc.tensor.matmul(y2[:, sl],
                                     w2tA[:, 128 * k:128 * k + 128],
                                     rhs, start=(k == 0), stop=False)
                    nc.tensor.matmul(y2[:, sl],
                                     w2tB[:, 128 * k:128 * k + 128],
                                     rhs, start=False, stop=False)
                nc.tensor.matmul(y2[:, sl], _bc(w2s[:], CONV_DT),
                                 _bc(_pi_rhs(m2l[:], 65, h), CONV_DT),
                                 start=False, stop=True)
            m3 = sp.tile([128, 1024], F32, tag="m3")
            nc.scalar.activation(m3[:], y2[:], AF.Relu)
            tap("m3_%d" % s, m3[:])
            if stage < 5:
                continue

            # ================= layer B prep =================
            m3r = sp.tile([128, 1024], F32, tag="m3r")
            nc.scalar.activation(m3r[:], y2[:], AF.Relu, scale=2.0)
            msq3 = sp.tile([128, 1024], F32, tag="msq3")
            nc.scalar.activation(msq3[:], m3[:], AF.Square)
            nsq3p = ppa.tile([1, 1024], F32, tag="acc")
            for cch in range(2):
                nc.tensor.matmul(nsq3p[:1, 512 * cch:512 * cch + 512],
                                 _bc(ones128[:], CONV_DT),
                                 _bc(msq3[:, 512 * cch:512 * cch + 512], CONV_DT),
                                 start=True, stop=True)
            nsq3n = sp.tile([1, 1024], F32, tag="nsq3")
            nc.scalar.activation(nsq3n[:], nsq3p[:1, :], AF.Copy, scale=-1.0)

            ttp3 = ppA.tile([128, 1024], F32, tag="A")
            for t in range(8):
                nc.tensor.matmul(ttp3[:, 128 * t:128 * t + 128],
                                 m3[:, 128 * t:128 * t + 128],
                                 ident[:], is_transpose=True,
                                 start=True, stop=True)
            m3T = sp.tile([128, 8, 128], BF16, tag="m3T")
            nc.scalar.activation(m3T[:].rearrange("p r c -> p (r c)"),
                                 ttp3[:], AF.Copy)
            tap("m3T_%d" % s, m3T[:])

            # ================= layer B kNN =================
            idx3 = sp.tile([128, 8, 8], U16, tag="idx3")   # [p][t][k]
            vals3 = sp.tile([128, 8, 8], F32, tag="vals3")
            for t in range(8):
                A = ppA.tile([128, 1024], F32, tag="A")
                for cch in range(2):
                    sl = slice(512 * cch, 512 * cch + 512)
                    nc.tensor.matmul(A[:, sl],
                                     _bc(m3[:, 128 * t:128 * t + 128], DIST_DT),
                                     _bc(m3r[:, sl], DIST_DT), start=True, stop=False)
                    nc.tensor.matmul(A[:, sl], _bc(onesr[:], DIST_DT),
                                     _bc(nsq3n[:, sl], DIST_DT),
                                     start=False, stop=True)
                nc.vector.tensor_add(A[:, 128 * t:128 * t + 128],
                                     A[:, 128 * t:128 * t + 128], diag[:])
                nc.vector.max(vals3[:, t, :], A[:])
                nc.vector.max_index(idx3[:, t, :], vals3[:, t, :], A[:])
            tap("idx3_%d" % s, idx3[:])
            if stage < 6:
                continue

            idxv3 = sp.tile([128, 64], U16, tag="idxv3")
            nc.vector.tensor_copy(
                idxv3[:].rearrange("p (k t) -> p k t", k=8),
                idx3[:].rearrange("p t k -> p k t"))
            w3idx = _emit_idx_table(nc, dp, sp, idxv3[:], "i3")
            g3 = gp.tile([128, 8192], BF16, tag="g3")
            nc.gpsimd.dma_gather(
                g3[:].rearrange("p (o n) -> p o n", o=1),
                m3T[:].rearrange("p r c -> p (r c)"),
                _bc(w3idx[:], I16), 8192, 8192, 128,
                transpose=True, single_packet=False,
                sbuf_tokens_per_rank=128, sbuf_free_dim_per_rank=256)
            tap("g3_%d" % s, g3[:])
            if stage < 7:
                continue

            # y3 = W3self@m3(pi-order) + sum_k W3k @ g3[tap k]  (+ bias)
            y3p = ppa.tile([128, 1024], F32, tag="acc")
            for h in range(2):
                sl = slice(512 * h, 512 * h + 512)
                for k in range(8):
                    rhs = _ap(g3[:], 4096 * h + 128 * k,
                              [[8192, 128], [1024, 4], [1, 128]])
                    nc.tensor.matmul(y3p[:12, sl],
                                     w3t[:, 12 * k:12 * k + 12],
                                     rhs, start=(k == 0), stop=False)
                nc.tensor.matmul(y3p[:12, sl], _bc(w3s[:], CONV_DT),
                                 _bc(_pi_rhs(m3[:], 128, h), CONV_DT),
                                 start=False, stop=True)
            y3sb = sp.tile([12, 1024], F32, tag="y3")
            nc.scalar.activation(y3sb[:], y3p[:12, :], AF.Identity, bias=b3c[:])
            tap("y3_%d" % s, y3sb[:])

            # contiguous store; pixel_shuffle happens on the host
            nc.sync.dma_start(out_d[s], y3sb[:])

    return nc


_CACHE = {}


def _get_compiled():
    if 'nc' not in _CACHE:
        nc = bacc.Bacc("TRN2", target_bir_lowering=False, debug=False,
                       num_devices=N_CORES)
        build_program(nc, SAMPLES)
        nc.compile()
        _CACHE['nc'] = nc
    return _CACHE['nc']


def make_in_maps(x, consts):
    in_maps = []
    for c in range(N_CORES):
        shard = np.ascontiguousarray(x[c * SAMPLES:(c + 1) * SAMPLES],
                                     dtype=np.float32)
        m = dict(consts)
        m['p27'] = build_p27(shard).astype(np.float32)
        in_maps.append(m)
    return in_maps


def kernel(x, conv1_w, conv1_b, conv2_w, conv2_b, conv3_w, conv3_b, **_ignored):
    x = np.asarray(x, np.float32)
    consts = build_consts(conv1_w, conv1_b, conv2_w, conv2_b, conv3_w, conv3_b)
    nc = _get_compiled()
    in_maps = make_in_maps(x, consts)
    res = bass_utils.run_bass_kernel_spmd(nc, in_maps, core_ids=list(range(N_CORES)))
    y3 = np.concatenate([res.results[c]['out'] for c in range(N_CORES)], axis=0)
    return shuffle_out(y3)


def shuffle_out(y3):
    """y3 (B, 12, 1024) with channel rows co' = q*3+ch -> (B, 3, 64, 64)."""
    B = y3.shape[0]
    y = y3.reshape(B, 4, 3, 32, 32)                # [b][q=(sy,sx)][ch][h][w]
    out = np.zeros((B, 3, 64, 64), np.float32)
    for q in range(4):
        sy, sx = q >> 1, q & 1
        out[:, :, sy::2, sx::2] = y[:, q]
    return out.astype(np.float32)


if __name__ == '__main__':
    nc = _get_compiled()
    print("compiled ok")


# revision 19
# speedup vs baseline: 3.6584x; 2.1400x over previous
"""Trainium2 Bass kernel for nn_DenoisingLocal_Global_ConvNN_2D (v2).

Network (per sample):
  conv3x3(3->16, pad 1) + ReLU
  -> pixel_unshuffle(2): m2 (64, 1024)  [tokens = 32x32 grid]
  -> kNN layer A: all-pairs dist on m2, top-9 (self rank 0),
     y2 = W2_0 @ m2 + sum_{k=1..8} W2_k @ m2[:, idx_k] + b2, ReLU -> m3 (128,1024)
  -> kNN layer B on m3: y3 = W3_0 @ m3 + sum_k W3_k @ m3[:, idx_k] + b3 (12,1024)
  -> pixel_shuffle(2) -> (3, 64, 64)

v2 design vs v1:
  * SBUF-source transposed dma_gather (tokens_per_rank=128): source layout =
    PE-transpose of the feature matrix (token c at partition c%128, rank
    c//128), gathered output is directly feature-major [ch, slot] bf16 --
    no DRAM round trip, no per-pair PE un-transposes.
  * Layer A gathers hi/lo bf16 split of m2 (exact to ~2^-18); layer B gathers
    plain bf16 m3.
  * Wrapped idx table build keeps the 3-DMA chain but with a slot order
    (slot = 1024a + 128k + 16t + b for token n=128t+16a+b) that makes every
    DMA hop move 128-byte-contiguous runs (v1's middle hop was 2-byte runs).
    Side effect: y2/m3 columns are in pi(n)=128a+16t+b order (an involution);
    layer B runs entirely in pi-space and y3 comes out in ORIGINAL order.
  * All matmuls fp32r (1 cyc/col at N=512) or bf16 instead of fp32 (4 cyc).

Sharding: pure data parallelism, 8 samples per NeuronCore x 8 cores.
"""
import sys

for _p in ('/opt/trn_rl_repo',):
    if _p not in sys.path:
        sys.path.insert(0, _p)

import numpy as np
import ml_dtypes
from contextlib import ExitStack

import concourse.bass as bass
import concourse.tile as tile
from concourse import bacc, mybir
from concourse import bass_utils

F32 = mybir.dt.float32
F32R = mybir.dt.float32r
F16 = mybir.dt.float16
U16 = mybir.dt.uint16
I16 = mybir.dt.int16
AF = mybir.ActivationFunctionType
NPF16 = np.float16

N_CORES = 8
SAMPLES = 8          # samples per core
NEG_BIG = -3.0e38

DIST_DT = F32        # distance matmuls (ranking needs full fp32)
CONV_DT = F32        # fp32-data conv matmuls (feed the ranking path)


def _bc(ap, dt):
    if ap.dtype == dt:
        return ap
    return ap.bitcast(dt)


# ----------------------------------------------------------------------------
# host-side input preparation (numpy)
# ----------------------------------------------------------------------------

def build_consts(w1, b1, w2, b2, w3, b3):
    w1 = np.asarray(w1, np.float32).reshape(16, 3, 3, 3)
    b1 = np.asarray(b1, np.float32)
    w2 = np.asarray(w2, np.float32).reshape(128, 64, 9)
    b2 = np.asarray(b2, np.float32)
    w3 = np.asarray(w3, np.float32).reshape(12, 128, 9)
    b3 = np.asarray(b3, np.float32)

    # conv1 lhsT: 4 phases, K=28 (27 taps + bias row), M=64 (16 ch x 4 phases)
    c1 = np.zeros((4, 28, 64), np.float32)
    for q in range(4):
        for dy in range(3):
            for dx in range(3):
                c1[q, np.arange(3)[:, None] * 9 + dy * 3 + dx,
                   np.arange(16)[None, :] * 4 + q] = w1[:, :, dy, dx].T
        c1[q, 27, np.arange(16) * 4 + q] = b1
    c1 = np.ascontiguousarray(c1.transpose(1, 0, 2).reshape(28, 256))

    # W2 neighbor taps, hi/lo split for full fp32 weight precision.
    # wh/wl bf16 [128, 8*128]: tap k block k-1; rows 0..63 == rows 64..127
    # (gathered elements are [hi|lo] splits, both halves see the same W part)
    w2f = np.stack([w2[:, :, k].T for k in range(1, 9)], 0)   # (8, 64, 128)
    w2hi = w2f.astype(NPF16).astype(np.float32)
    w2lo = (w2f - w2hi).astype(NPF16)
    w2hi = w2hi.astype(NPF16)
    w2tapsA = np.zeros((128, 8, 128), NPF16)
    w2tapsB = np.zeros((128, 8, 128), NPF16)
    for k in range(8):
        w2tapsA[:64, k] = w2hi[k]
        w2tapsA[64:, k] = w2hi[k]
        w2tapsB[:64, k] = w2lo[k]
        w2tapsB[64:, k] = w2lo[k]
    w2tapsA = w2tapsA.reshape(128, 1024)
    w2tapsB = w2tapsB.reshape(128, 1024)
    w2self = np.zeros((65, 128), np.float32)
    w2self[:64] = w2[:, :, 0].T
    w2self[64] = b2

    # output-channel permutation: co = ch*4+q -> co' = q*3+ch so each
    # pixel_shuffle phase q reads contiguous partitions [3q:3q+3]
    perm = np.zeros(12, np.int64)
    for ch in range(3):
        for q in range(4):
            perm[q * 3 + ch] = ch * 4 + q
    w3 = w3[perm]
    b3 = b3[perm]

    # W3 neighbor taps, bf16 [128, 8*12]
    w3taps = np.zeros((128, 8, 12), np.float32)
    for k in range(1, 9):
        w3taps[:, k - 1] = w3[:, :, k].T
    w3taps = w3taps.reshape(128, 96).astype(NPF16)
    w3self = np.ascontiguousarray(w3[:, :, 0].T)          # (128, 12)
    b3col = np.ascontiguousarray(b3[:, None])             # (12, 1)

    ident = np.eye(128, dtype=np.float32)
    diagneg = np.zeros((128, 128), np.float32)
    np.fill_diagonal(diagneg, NEG_BIG)

    return dict(c1=c1, w2tapsA=w2tapsA, w2tapsB=w2tapsB, w2self=w2self,
                w3taps=w3taps, w3self=w3self, b3col=b3col, ident=ident,
                diagneg=diagneg)


def build_p27(x_shard):
    """Per-phase im2col for conv1: (S, 4, 28, 1024).
    p27[s, q=(sy,sx), 9ci+3dy+dx, 32y+x] = xpad[s, ci, 2y+sy+dy, 2x+sx+dx];
    row 27 = 1.0 (bias)."""
    S = x_shard.shape[0]
    xp = np.zeros((S, 3, 66, 66), np.float32)
    xp[:, :, 1:65, 1:65] = x_shard
    p27 = np.ones((S, 4, 28, 1024), np.float32)
    for q in range(4):
        sy, sx = q >> 1, q & 1
        for ci in range(3):
            for dy in range(3):
                for dx in range(3):
                    v = xp[:, ci, sy + dy:sy + dy + 64:2, sx + dx:sx + dx + 64:2]
                    p27[:, q, ci * 9 + dy * 3 + dx, :] = v.reshape(S, 1024)
    return p27


# ----------------------------------------------------------------------------
# device program
# ----------------------------------------------------------------------------

def _ap(base_ap, offset, dims):
    return bass.AP(base_ap.tensor, offset, [list(d) for d in dims])


def _emit_idx_table(nc, dp, sp, idxv, tag):
    """idxv: SBUF (128, 64) u16, value at [p=16a+b, m] is the index for
    slot s = 16*(64a + m) + b.  Produces the wrapped dma_gather table
    (128, 512) u16: W[16r + b, j] = value(slot 16j + b), j = 64a + m.
    All three DMA hops move >=128B-contiguous runs."""
    da = dp.tile([8192], U16, tag=tag + "a")
    # dump: da flat = p*64 + m = 1024a + 64b + m
    nc.sync.dma_start(da[:], idxv)
    db = dp.tile([8192], U16, tag=tag + "b")
    # shuffle: db[b*512 + 64a + m] <- da[1024a + 64b + m]  (64-elem runs)
    nc.sync.dma_start(
        _ap(db[:], 0, [[512, 16], [64, 8], [1, 64]]),
        _ap(da[:], 0, [[64, 16], [1024, 8], [1, 64]]))
    w = sp.tile([128, 512], U16, tag=tag + "w")
    # replicate rows b -> 16r+b
    nc.sync.dma_start(
        w[:], _ap(db[:], 0, [[0, 8], [512, 16], [1, 512]]))
    return w


def _pi_rhs(t_ap, part, ncols_off_h):
    """rhs AP streaming columns of a [part, 1024] SBUF tile in pi order
    (a, t, b) for cch half h: offset 16*(4h), dims a(4),t(8),b(16)."""
    h = ncols_off_h
    return _ap(t_ap, 16 * 4 * h,
               [[1024, part], [16, 4], [128, 8], [1, 16]])


def build_program(nc, samples=SAMPLES, tap=None, stage=99, repeat=1):
    p27_d = nc.dram_tensor("p27", (samples, 4, 28, 1024), F32, kind="ExternalInput").ap()
    c1_d = nc.dram_tensor("c1", (28, 256), F32, kind="ExternalInput").ap()
    w2tA_d = nc.dram_tensor("w2tapsA", (128, 1024), F16, kind="ExternalInput").ap()
    w2tB_d = nc.dram_tensor("w2tapsB", (128, 1024), F16, kind="ExternalInput").ap()
    w2s_d = nc.dram_tensor("w2self", (65, 128), F32, kind="ExternalInput").ap()
    w3t_d = nc.dram_tensor("w3taps", (128, 96), F16, kind="ExternalInput").ap()
    w3s_d = nc.dram_tensor("w3self", (128, 12), F32, kind="ExternalInput").ap()
    b3_d = nc.dram_tensor("b3col", (12, 1), F32, kind="ExternalInput").ap()
    id_d = nc.dram_tensor("ident", (128, 128), F32, kind="ExternalInput").ap()
    dg_d = nc.dram_tensor("diagneg", (128, 128), F32, kind="ExternalInput").ap()
    out_d = nc.dram_tensor("out", (samples, 12, 1024), F32, kind="ExternalOutput").ap()

    if tap is None:
        def tap(name, t):
            pass

    with tile.TileContext(nc) as tc, ExitStack() as ctx:
        cp = ctx.enter_context(tc.tile_pool(name="consts", bufs=1))
        sp = ctx.enter_context(tc.tile_pool(name="sb", bufs=2))
        gp = ctx.enter_context(tc.tile_pool(name="gm", bufs=2))
        ppA = ctx.enter_context(tc.tile_pool(name="psA", bufs=2, space="PSUM"))
        ppa = ctx.enter_context(tc.tile_pool(name="psacc", bufs=2, space="PSUM"))
        dp = ctx.enter_context(tc.tile_pool(name="dram", bufs=2, space="DRAM"))

        c1 = cp.tile([28, 256], F32); nc.sync.dma_start(c1[:], c1_d)
        w2tA = cp.tile([128, 1024], F16); nc.sync.dma_start(w2tA[:], w2tA_d)
        w2tB = cp.tile([128, 1024], F16); nc.sync.dma_start(w2tB[:], w2tB_d)
        w2s = cp.tile([65, 128], F32); nc.sync.dma_start(w2s[:], w2s_d)
        w3t = cp.tile([128, 96], F16); nc.sync.dma_start(w3t[:], w3t_d)
        w3s = cp.tile([128, 12], F32); nc.sync.dma_start(w3s[:], w3s_d)
        b3c = cp.tile([12, 1], F32); nc.sync.dma_start(b3c[:], b3_d)
        ident = cp.tile([128, 128], F32); nc.sync.dma_start(ident[:], id_d)
        diag = cp.tile([128, 128], F32); nc.sync.dma_start(diag[:], dg_d)
        ones64 = cp.tile([64, 1], F32); nc.gpsimd.memset(ones64[:], 1.0)
        ones128 = cp.tile([128, 1], F32); nc.gpsimd.memset(ones128[:], 1.0)
        ones1h = cp.tile([1, 128], F16); nc.gpsimd.memset(ones1h[:], 1.0)

        for s in [ss for _ in range(repeat) for ss in range(samples)]:
            # ================= conv1 =================
            p27 = sp.tile([28, 4096], F32, tag="p27")
            nc.sync.dma_start(p27[:].rearrange("p (q n) -> p q n", q=4),
                              p27_d[s].rearrange("q p n -> p q n"))
            H = ppa.tile([128, 1024], F32, tag="acc")
            for cch in range(2):
                for q in range(4):
                    rhs = p27[:, 1024 * q + 512 * cch:1024 * q + 512 * cch + 512]
                    nc.tensor.matmul(H[:64, 512 * cch:512 * cch + 512],
                                     _bc(c1[:, 64 * q:64 * q + 64], CONV_DT),
                                     _bc(rhs, CONV_DT), start=(q == 0), stop=(q == 3))
            m2l = sp.tile([65, 1024], F32, tag="m2l")
            nc.scalar.activation(m2l[:64, :], H[:64, :], AF.Relu)
            nc.gpsimd.memset(m2l[64:65, :], 1.0)
            m2r = sp.tile([65, 1024], F32, tag="m2r")
            nc.scalar.activation(m2r[:64, :], H[:64, :], AF.Relu, scale=2.0)
            msq = sp.tile([64, 1024], F32, tag="msq")
            nc.scalar.activation(msq[:], m2l[:64, :], AF.Square)
            nsqp = ppa.tile([1, 1024], F32, tag="acc")
            for cch in range(2):
                nc.tensor.matmul(nsqp[:1, 512 * cch:512 * cch + 512],
                                 _bc(ones64[:], CONV_DT),
                                 _bc(msq[:, 512 * cch:512 * cch + 512], CONV_DT),
                                 start=True, stop=True)
            nc.scalar.activation(m2r[64:65, :], nsqp[:1, :], AF.Copy, scale=-1.0)
            tap("m2l_%d" % s, m2l[:])
            tap("m2r_%d" % s, m2r[:])

            # m2T: token-major gather source [p, rank, 128] bf16 = [hi | lo]
            ttp = ppA.tile([128, 1024], F32, tag="A")
            for t in range(8):
                nc.tensor.matmul(ttp[:, 64 * t:64 * t + 64],
                                 m2l[:64, 128 * t:128 * t + 128],
                                 ident[:64, :64], is_transpose=True,
                                 start=True, stop=True)
            m2T = sp.tile([128, 8, 128], F16, tag="m2T")
            nc.scalar.activation(
                _ap(m2T[:], 0, [[1024, 128], [128, 8], [1, 64]]),
                ttp[:, :512], AF.Copy)
            nc.vector.tensor_sub(
                _ap(m2T[:], 64, [[1024, 128], [128, 8], [1, 64]]),
                ttp[:, :512],
                _ap(m2T[:], 0, [[1024, 128], [128, 8], [1, 64]]))
            tap("m2T_%d" % s, m2T[:])
            if stage < 2:
                continue

            # ================= layer A kNN =================
            idxn = sp.tile([128, 8, 8], U16, tag="idxn")   # [p][t][k]
            vals2 = sp.tile([128, 8, 8], F32, tag="vals2")
            for t in range(8):
                A = ppA.tile([128, 1024], F32, tag="A")
                for cch in range(2):
                    nc.tensor.matmul(A[:, 512 * cch:512 * cch + 512],
                                     _bc(m2l[:, 128 * t:128 * t + 128], DIST_DT),
                                     _bc(m2r[:, 512 * cch:512 * cch + 512], DIST_DT),
                                     start=True, stop=True)
                nc.vector.tensor_add(A[:, 128 * t:128 * t + 128],
                                     A[:, 128 * t:128 * t + 128], diag[:])
                nc.vector.max(vals2[:, t, :], A[:])
                nc.vector.max_index(idxn[:, t, :], vals2[:, t, :], A[:])
            tap("idxn_%d" % s, idxn[:])
            if stage < 3:
                continue

            # idxv[p, m=8k+t] = idxn[p, t, k]
            idxv2 = sp.tile([128, 64], U16, tag="idxv2")
            nc.vector.tensor_copy(
                idxv2[:].rearrange("p (k t) -> p k t", k=8),
                idxn[:].rearrange("p t k -> p k t"))
            w2idx = _emit_idx_table(nc, dp, sp, idxv2[:], "i2")
            g2 = gp.tile([128, 8192], F16, tag="g2")
            nc.gpsimd.dma_gather(
                g2[:].rearrange("p (o n) -> p o n", o=1),
                m2T[:].rearrange("p r c -> p (r c)"),
                _bc(w2idx[:], I16), 8192, 8192, 128,
                transpose=True, single_packet=False,
                sbuf_tokens_per_rank=128, sbuf_free_dim_per_rank=256)
            tap("g2_%d" % s, g2[:])
            if stage < 4:
                continue

            # y2 = W2self@m2l(pi-order) + sum_k W2k @ g2[tap k]
            y2 = ppa.tile([128, 1024], F32, tag="acc")
            for h in range(2):
                sl = slice(512 * h, 512 * h + 512)
                for k in range(8):
                    rhs = _ap(g2[:], 4096 * h + 128 * k,
                              [[8192, 128], [1024, 4], [1, 128]])
                    nc.tensor.matmul(y2[:, sl],
                                     w2tA[:, 128 * k:128 * k + 128],
                                     rhs, start=(k == 0), stop=False)
                    nc.tensor.matmul(y2[:, sl],
                                     w2tB[:, 128 * k:128 * k + 128],
                                     rhs, start=False, stop=False)
                nc.tensor.matmul(y2[:, sl], _bc(w2s[:], CONV_DT),
                                 _bc(_pi_rhs(m2l[:], 65, h), CONV_DT),
                                 start=False, stop=True)
            m3 = sp.tile([128, 1024], F32, tag="m3")
            nc.scalar.activation(m3[:], y2[:], AF.Relu)
            tap("m3_%d" % s, m3[:])
            if stage < 5:
                continue

            # ================= layer B prep =================
            m3r = sp.tile([128, 1024], F32, tag="m3r")
            nc.scalar.activation(m3r[:], y2[:], AF.Relu, scale=2.0)
            msq3 = sp.tile([128, 1024], F32, tag="msq3")
            nc.scalar.activation(msq3[:], m3[:], AF.Square)
            nsq3p = ppa.tile([1, 1024], F32, tag="acc")
            for cch in range(2):
                nc.tensor.matmul(nsq3p[:1, 512 * cch:512 * cch + 512],
                                 _bc(ones128[:], CONV_DT),
                                 _bc(msq3[:, 512 * cch:512 * cch + 512], CONV_DT),
                                 start=True, stop=True)
            # -nsq as an fp16 hi/lo pair (exact to ~2^-22), both at partition 0
            nsqhi = sp.tile([1, 1024], F16, tag="nsqhi")
            nc.scalar.activation(nsqhi[:], nsq3p[:1, :], AF.Copy, scale=-1.0)
            nsqlo = sp.tile([1, 1024], F16, tag="nsqlo")
            nc.vector.scalar_tensor_tensor(
                nsqlo[:], nsq3p[:1, :], -1.0, nsqhi[:],
                mybir.AluOpType.mult, mybir.AluOpType.subtract)

            ttp3 = ppA.tile([128, 1024], F32, tag="A")
            for t in range(8):
                nc.tensor.matmul(ttp3[:, 128 * t:128 * t + 128],
                                 m3[:, 128 * t:128 * t + 128],
                                 ident[:], is_transpose=True,
                                 start=True, stop=True)
            m3T = sp.tile([128, 8, 128], F16, tag="m3T")
            nc.scalar.activation(m3T[:].rearrange("p r c -> p (r c)"),
                                 ttp3[:], AF.Copy)
            tap("m3T_%d" % s, m3T[:])

            # ================= layer B kNN =================
            idx3 = sp.tile([128, 8, 8], U16, tag="idx3")   # [p][t][k]
            vals3 = sp.tile([128, 8, 8], F32, tag="vals3")
            for t in range(8):
                A = ppA.tile([128, 1024], F32, tag="A")
                for cch in range(2):
                    sl = slice(512 * cch, 512 * cch + 512)
                    nc.tensor.matmul(A[:, sl],
                                     _bc(m3[:, 128 * t:128 * t + 128], DIST_DT),
                                     _bc(m3r[:, sl], DIST_DT), start=True, stop=False)
                    nc.tensor.matmul(A[:, sl], ones1h[:], nsqhi[:, sl],
                                     start=False, stop=False)
                    nc.tensor.matmul(A[:, sl], ones1h[:], nsqlo[:, sl],
                                     start=False, stop=True)
                nc.vector.tensor_add(A[:, 128 * t:128 * t + 128],
                                     A[:, 128 * t:128 * t + 128], diag[:])
                nc.vector.max(vals3[:, t, :], A[:])
                nc.vector.max_index(idx3[:, t, :], vals3[:, t, :], A[:])
            tap("idx3_%d" % s, idx3[:])
            if stage < 6:
                continue

            idxv3 = sp.tile([128, 64], U16, tag="idxv3")
            nc.vector.tensor_copy(
                idxv3[:].rearrange("p (k t) -> p k t", k=8),
                idx3[:].rearrange("p t k -> p k t"))
            w3idx = _emit_idx_table(nc, dp, sp, idxv3[:], "i3")
            g3 = gp.tile([128, 8192], F16, tag="g3")
            nc.gpsimd.dma_gather(
                g3[:].rearrange("p (o n) -> p o n", o=1),
                m3T[:].rearrange("p r c -> p (r c)"),
                _bc(w3idx[:], I16), 8192, 8192, 128,
                transpose=True, single_packet=False,
                sbuf_tokens_per_rank=128, sbuf_free_dim_per_rank=256)
            tap("g3_%d" % s, g3[:])
            if stage < 7:
                continue

            # y3 = W3self@m3(pi-order) + sum_k W3k @ g3[tap k]  (+ bias)
            y3p = ppa.tile([128, 1024], F32, tag="acc")
            for h in range(2):
                sl = slice(512 * h, 512 * h + 512)
                for k in range(8):
                    rhs = _ap(g3[:], 4096 * h + 128 * k,
                              [[8192, 128], [1024, 4], [1, 128]])
                    nc.tensor.matmul(y3p[:12, sl],
                                     w3t[:, 12 * k:12 * k + 12],
                                     rhs, start=(k == 0), stop=False)
                nc.tensor.matmul(y3p[:12, sl], _bc(w3s[:], CONV_DT),
                                 _bc(_pi_rhs(m3[:], 128, h), CONV_DT),
                                 start=False, stop=True)
            y3sb = sp.tile([12, 1024], F32, tag="y3")
            nc.scalar.activation(y3sb[:], y3p[:12, :], AF.Identity, bias=b3c[:])
            tap("y3_%d" % s, y3sb[:])

            # contiguous store; pixel_shuffle happens on the host
            nc.sync.dma_start(out_d[s], y3sb[:])

    return nc


_CACHE = {}


def _get_compiled():
    if 'nc' not in _CACHE:
        nc = bacc.Bacc("TRN2", target_bir_lowering=False, debug=False,
                       num_devices=N_CORES)
        build_program(nc, SAMPLES)
        nc.compile()
        _CACHE['nc'] = nc
    return _CACHE['nc']


def make_in_maps(x, consts):
    in_maps = []
    for c in range(N_CORES):
        shard = np.ascontiguousarray(x[c * SAMPLES:(c + 1) * SAMPLES],
                                     dtype=np.float32)
        m = dict(consts)
        m['p27'] = build_p27(shard).astype(np.float32)
        in_maps.append(m)
    return in_maps


def kernel(x, conv1_w, conv1_b, conv2_w, conv2_b, conv3_w, conv3_b, **_ignored):
    x = np.asarray(x, np.float32)
    consts = build_consts(conv1_w, conv1_b, conv2_w, conv2_b, conv3_w, conv3_b)
    nc = _get_compiled()
    in_maps = make_in_maps(x, consts)
    res = bass_utils.run_bass_kernel_spmd(nc, in_maps, core_ids=list(range(N_CORES)))
    y3 = np.concatenate([res.results[c]['out'] for c in range(N_CORES)], axis=0)
    return shuffle_out(y3)


def shuffle_out(y3):
    """y3 (B, 12, 1024) with channel rows co' = q*3+ch -> (B, 3, 64, 64)."""
    B = y3.shape[0]
    y = y3.reshape(B, 4, 3, 32, 32)                # [b][q=(sy,sx)][ch][h][w]
    out = np.zeros((B, 3, 64, 64), np.float32)
    for q in range(4):
        sy, sx = q >> 1, q & 1
        out[:, :, sy::2, sx::2] = y[:, q]
    return out.astype(np.float32)


if __name__ == '__main__':
    nc = _get_compiled()
    print("compiled ok")
